# revision 22
# baseline (speedup 1.0000x reference)
"""Trainium2 Bass kernel for nn_MHAEncoderFusedProj.

B=4, S=2048, E=1024, H=16, D=64. Sharding: 8 cores = 4 batch x 2
head-groups (8 heads each). No collectives: each core computes a partial
out-projection over its 512 o-features; the host adds the two partials per
batch element and transposes back.

v2 design (all-bf16 data path, ACT-paced attention with PE fillers):
  - All matmul operands are bf16 (1 col/cycle PE streaming vs ~2 for
    fp32/fp32r). PSUM accumulation stays fp32.
  - x^T is loaded into SBUF once (bf16) and reused by the QKV projections.
  - Attention inner loop per (pair, q-chunk, kt): score pair packed into PE
    row groups (runs concurrently), exp on ScalarE (the pacer at ~1.1us per
    [128,1024] tile), PV accumulation with a ones-column appended to V so
    row 64 is the softmax denominator.
  - The ScalarE exp stream leaves ~300-400ns/iter of PE slack; projection,
    RoPE and out-projection matmuls for the *next* head-pair are emitted as
    fine-grained "fillers" (one matmul per kt slot) so they hide inside the
    attention window instead of extending the span.
  - Normalization: reciprocal of the denominator row, gpsimd partition
    broadcast, DVE multiply (second head's half moved by SBUF-SBUF DMA).
"""

import math
import os

import numpy as np

P = 128
D = 64

FULL_CFG = dict(S=2048, E=1024, HG=8)


def _emit(nc, tc, io, cfg):
    import concourse.mybir as mybir

    FP32 = mybir.dt.float32
    BF16 = mybir.dt.bfloat16
    EXP = mybir.ActivationFunctionType.Exp
    CPY = mybir.ActivationFunctionType.Copy

    S, E, HG = cfg["S"], cfg["E"], cfg["HG"]
    EO = E // P              # contraction tiles over embedding
    MQK = 2 * HG * D // P    # Q+K feature tiles (2 heads per tile)
    NPAIR = HG // 2          # head pairs per core
    FV = HG * D              # V features
    KT = S // P              # key token tiles
    CH = 512                 # chunk for projections / rope / out-proj
    NTA = S // CH
    QCH = 512                # q chunk in attention
    NQI = S // QCH
    FO = E // P              # out-proj feature tiles
    EOV = FV // P            # out-proj contraction tiles
    scale = 1.0 / math.sqrt(D)
    PUMP_QI = cfg.get("pump_qi", 8)
    PUMP_HP3 = cfg.get("pump_hp3", 2)

    xT = io["xT"].ap()          # [E, S] bf16
    wqkT = io["wqkT"].ap()      # [E, 2*HG*D] bf16
    wvT = io["wvT"].ap()        # [E, HG*D] bf16
    woutT = io["woutT"].ap()    # [HG*D, E] bf16
    cos2T = io["cos2T"].ap()    # [P, S] bf16
    sin2T = io["sin2T"].ap()    # [P, S] fp32
    p2 = io["p2"].ap()          # [P, P] bf16 signed rotate-half permutation
    ones = io["ones"]           # [P, KT*HG] bf16
    outT = io["outT"].ap()      # [E, S] bf16

    xT_t = xT.rearrange("(eo p) t -> p eo t", p=P)
    outT_t = outT.rearrange("(fo p) t -> p fo t", p=P)

    from contextlib import ExitStack

    with ExitStack() as top:
        persist = top.enter_context(tc.tile_pool(name="persist", bufs=1))
        # PSUM budget (8 banks): scores ping/pong 2x[128,1024] = 4 banks,
        # PV accumulators 2x[128,512] = 2 banks, filler (proj/rope/B/D)
        # 2x[128,512] = 2 banks.
        psc = top.enter_context(tc.tile_pool(name="psc", bufs=2, space="PSUM"))
        ppv = top.enter_context(tc.tile_pool(name="ppv", bufs=1, space="PSUM"))
        pfill = top.enter_context(tc.tile_pool(name="pfill", bufs=1, space="PSUM"))
        prope = top.enter_context(tc.tile_pool(name="prope", bufs=1, space="PSUM"))
        ep = top.enter_context(tc.tile_pool(name="ep", bufs=4))
        tap = top.enter_context(tc.tile_pool(name="ropeT", bufs=1))
        npool = top.enter_context(tc.tile_pool(name="norm", bufs=2))
        opool = top.enter_context(tc.tile_pool(name="opool", bufs=2))
        dcp = top.enter_context(tc.tile_pool(name="dcp", bufs=2))

        xsb = persist.tile([P, EO, S], BF16, tag="xsb")
        wqk = persist.tile([P, EO, MQK * P], BF16, tag="wqk")
        wv = persist.tile([P, EO, FV], BF16, tag="wv")
        wo = persist.tile([P, EOV, E], BF16, tag="wo")
        vsb = persist.tile([P, KT, HG, D + 1], BF16, tag="vsb")
        qk = [persist.tile([P, S], BF16, tag=f"qk{m}", name=f"qk{m}") for m in range(MQK)]
        ost = [persist.tile([P, S], BF16, tag=f"ost{j}", name=f"ost{j}") for j in range(NPAIR)]
        cosb = persist.tile([P, S], BF16, tag="cosb")
        sinb = persist.tile([P, S], FP32, tag="sinb")
        p2b = persist.tile([P, P], BF16, tag="p2b")

        # upfront DMAs, split across the two HWDGE queues (sync + scalar):
        # x and wqk ride the scalar queue, everything else the sync queue.
        nc.sync.dma_start(wv, wvT.rearrange("(eo p) f -> p eo f", p=P))
        XD = 256
        for c in range(S // XD):
            eng = nc.scalar if c % 2 == 0 else nc.sync
            eng.dma_start(
                xsb[:, :, c * XD : (c + 1) * XD], xT_t[:, :, c * XD : (c + 1) * XD]
            )
        nc.scalar.dma_start(wqk, wqkT.rearrange("(eo p) f -> p eo f", p=P))
        nc.sync.dma_start(vsb[:, :, :, D : D + 1], ones.ap())
        nc.sync.dma_start(cosb, cos2T)
        nc.sync.dma_start(sinb, sin2T)
        nc.sync.dma_start(p2b, p2)
        nc.sync.dma_start(wo, woutT.rearrange("(eo p) f -> p eo f", p=P))

        # ---------------- filler machinery ----------------
        fill_q = []

        def pump(n):
            for _ in range(n):
                while fill_q:
                    try:
                        next(fill_q[0])()
                        break
                    except StopIteration:
                        fill_q.pop(0)
                else:
                    return

        def drain():
            while fill_q:
                try:
                    next(fill_q[0])()
                except StopIteration:
                    fill_q.pop(0)

        # ---------------- phase emitters ----------------
        def b_unit(tt, on_act=False):
            """V projection for token tile tt (all 8 heads)."""
            ps = pfill.tile([P, FV], FP32, tag="fill", name="psB")
            for e in range(EO):
                nc.tensor.matmul(
                    ps,
                    xsb[:, e, tt * P : (tt + 1) * P],
                    wv[:, e, :],
                    start=(e == 0),
                    stop=(e == EO - 1),
                )
            srcv = ps.rearrange("p (h d) -> p h d", d=D)
            if on_act:
                nc.scalar.activation(vsb[:, tt, :, 0:D], srcv, CPY)
            else:
                nc.vector.tensor_copy(vsb[:, tt, :, 0:D], srcv)

        def a_unit_gen(m, ta, on_act=False):
            """QK projection chunk + RoPE, one PE op per yield."""
            sl = slice(ta * CH, (ta + 1) * CH)
            ps = pfill.tile([P, CH], FP32, tag="fill", name="psA")
            for e in range(EO):
                yield (
                    lambda ps=ps, m=m, e=e, sl=sl: nc.tensor.matmul(
                        ps,
                        wqk[:, e, m * P : (m + 1) * P],
                        xsb[:, e, sl],
                        start=(e == 0),
                        stop=(e == EO - 1),
                    )
                )

            def cp(ps=ps, m=m, sl=sl):
                if on_act:
                    nc.scalar.activation(qk[m][:, sl], ps, CPY)
                else:
                    nc.vector.tensor_copy(qk[m][:, sl], ps)

            yield cp

            def rope(m=m, sl=sl):
                rps = prope.tile([P, CH], FP32, tag="rps", name="rps")
                nc.tensor.matmul(rps, p2b, qk[m][:, sl], start=True, stop=True)
                t1 = tap.tile([P, CH], BF16, tag="t1")
                nc.vector.tensor_mul(t1, qk[m][:, sl], cosb[:, sl])
                t2 = tap.tile([P, CH], BF16, tag="t2")
                nc.vector.tensor_mul(t2, rps, sinb[:, sl])
                nc.vector.tensor_add(qk[m][:, sl], t1, t2)

            yield rope

        def a_pair_gen(hp):
            """QK projection + RoPE for head pair hp, one PE op per yield."""
            for ta in range(NTA):
                for m in (hp, NPAIR + hp):
                    yield from a_unit_gen(m, ta, on_act=False)

        def d_gen(td):
            """Partial out-projection for token chunk td (needs all pairs)."""
            sl = slice(td * CH, (td + 1) * CH)
            for fo in range(FO):
                ps = pfill.tile([P, CH], FP32, tag="fill", name="psD")
                for e in range(EOV):
                    yield (
                        lambda ps=ps, fo=fo, e=e, sl=sl: nc.tensor.matmul(
                            ps,
                            wo[:, e, fo * P : (fo + 1) * P],
                            ost[e][:, sl],
                            start=(e == 0),
                            stop=(e == EOV - 1),
                        )
                    )

                def cpd(ps=ps, fo=fo, sl=sl):
                    ot = dcp.tile([P, CH], BF16, tag="ot")
                    nc.vector.tensor_copy(ot, ps)
                    nc.gpsimd.dma_start(outT_t[:, fo, sl], ot)

                yield cpd

        def attention_pair(hp, pump_n):
            qt = qk[hp]
            ktile = qk[NPAIR + hp]
            for qi in range(NQI):
                qsl = slice(qi * QCH, (qi + 1) * QCH)
                ops = [
                    ppv.tile([P, QCH], FP32, tag=f"pv{hs}", name=f"ops{hs}")
                    for hs in range(2)
                ]
                exs = {}

                def sc_exp(kt):
                    scps = psc.tile([P, 2 * QCH], FP32, tag="sc", name="scps")
                    ksl = slice(kt * P, (kt + 1) * P)
                    for hs in range(2):
                        b = hs * D
                        nc.tensor.matmul(
                            scps[:, hs * QCH : (hs + 1) * QCH],
                            ktile[b : b + D, ksl],
                            qt[b : b + D, qsl],
                            start=True,
                            stop=True,
                        )
                    ex = ep.tile([P, 2 * QCH], BF16, tag="exp")
                    nc.scalar.activation(ex, scps, EXP, scale=scale)
                    exs[kt] = ex

                def pv(kt):
                    ex = exs.pop(kt)
                    for hs in range(2):
                        nc.tensor.matmul(
                            ops[hs][0 : D + 1, :],
                            vsb[:, kt, 2 * hp + hs, :],
                            ex[:, hs * QCH : (hs + 1) * QCH],
                            start=(kt == 0),
                            stop=(kt == KT - 1),
                        )

                sc_exp(0)
                for kt in range(1, KT):
                    sc_exp(kt)
                    pump(pump_n)
                    pv(kt - 1)
                pv(KT - 1)

                # free the PV PSUM banks immediately, normalize lazily from SBUF
                ostas = []
                for hs in range(2):
                    osta = opool.tile([D + 1, QCH], FP32, tag=f"osta{hs}")
                    nc.vector.tensor_copy(osta, ops[hs][0 : D + 1, :])
                    ostas.append(osta)
                for hs in range(2):
                    osta = ostas[hs]
                    # transpose the denominator row to [128, 4] so the
                    # multi-pass reciprocal runs on a tiny free dim
                    rs8 = npool.tile([P, QCH // P], FP32, tag=f"rs8{hs}")
                    nc.sync.dma_start(rs8, osta[D : D + 1, :])
                    ri8 = npool.tile([P, QCH // P], FP32, tag=f"ri8{hs}")
                    nc.vector.reciprocal(ri8, rs8)
                    rr = npool.tile([1, QCH], FP32, tag=f"rr{hs}")
                    nc.sync.dma_start(rr, ri8)
                    rbc = npool.tile([D, QCH], FP32, tag=f"rbc{hs}")
                    nc.gpsimd.partition_broadcast(rbc, rr)
                    if hs == 0:
                        nc.gpsimd.tensor_mul(ost[hp][0:D, qsl], osta[0:D, :], rbc)
                    else:
                        otmp = npool.tile([D, QCH], BF16, tag="otmp")
                        nc.gpsimd.tensor_mul(otmp, osta[0:D, :], rbc)
                        nc.sync.dma_start(ost[hp][D : 2 * D, qsl], otmp)

                if hp == NPAIR - 1:
                    fill_q.append(d_gen(qi))
                pump(PUMP_QI)

        # ---------------- prologue: V proj + pair-0 QK/rope ----------------
        # Copies go on ScalarE (idle before attention) so the DVE never
        # develops a backlog that would stall attention-era filler matmuls.
        for c in range(NTA):
            for f in a_unit_gen(NPAIR, c, on_act=True):   # pair-0 K chunk first
                f()
            for f in a_unit_gen(0, c, on_act=True):       # pair-0 Q chunk
                f()
            for tt in range(4 * c, 4 * c + 4):
                b_unit(tt, on_act=True)

        # ---------------- attention with interleaved fillers ----------------
        for hp in range(NPAIR):
            if hp < NPAIR - 1:
                fill_q.append(a_pair_gen(hp + 1))
            attention_pair(hp, PUMP_HP3 if hp == NPAIR - 1 else 1)
            if hp < NPAIR - 1:
                drain()  # pair hp+1 must be fully projected+roped
        drain()  # remaining out-projection chunks


def _build(cfg):
    from concourse import bacc
    import concourse.mybir as mybir
    import concourse.tile as tile

    S, E, HG = cfg["S"], cfg["E"], cfg["HG"]
    FP32 = mybir.dt.float32
    BF16 = mybir.dt.bfloat16
    nc = bacc.Bacc("TRN2", target_bir_lowering=False, debug=False)
    io = {
        "xT": nc.dram_tensor("xT", [E, S], BF16, kind="ExternalInput"),
        "wqkT": nc.dram_tensor("wqkT", [E, 2 * HG * D], BF16, kind="ExternalInput"),
        "wvT": nc.dram_tensor("wvT", [E, HG * D], BF16, kind="ExternalInput"),
        "woutT": nc.dram_tensor("woutT", [HG * D, E], BF16, kind="ExternalInput"),
        "cos2T": nc.dram_tensor("cos2T", [P, S], BF16, kind="ExternalInput"),
        "sin2T": nc.dram_tensor("sin2T", [P, S], FP32, kind="ExternalInput"),
        "p2": nc.dram_tensor("p2", [P, P], BF16, kind="ExternalInput"),
        "ones": nc.dram_tensor(
            "ones", [P, (S // P) * HG], BF16, kind="ExternalInput"
        ),
        "outT": nc.dram_tensor("outT", [E, S], BF16, kind="ExternalOutput"),
    }
    with tile.TileContext(nc) as tc:
        _emit(nc, tc, io, cfg)
    nc.compile()
    return nc


def _rot_matrix():
    """P2[p, m] such that (P2^T @ v) = rotate_half(v) for the 2-head
    [128]-row layout (two independent 64-blocks)."""
    p2 = np.zeros((P, P), dtype=np.float32)
    for blk in (0, 64):
        for d in range(32):
            p2[blk + d + 32, blk + d] = -1.0
            p2[blk + d, blk + d + 32] = 1.0
    return p2


def make_core_inputs(x, cos, sin, W_qkv, W_out, cfg=FULL_CFG):
    """Host-side shard prep. Returns list of 8 in_maps."""
    import ml_dtypes

    bf = ml_dtypes.bfloat16
    S, E, HG = cfg["S"], cfg["E"], cfg["HG"]
    B = x.shape[0]
    NG = 2  # head groups
    FG = HG * D  # features per group
    cos2T = np.ascontiguousarray(np.tile(cos.T, (2, 1))).astype(bf)
    sin2T = np.ascontiguousarray(np.tile(sin.T, (2, 1))).astype(np.float32)
    p2 = _rot_matrix().astype(bf)
    ones = np.ones((P, (S // P) * HG), dtype=bf)
    xTs = [np.ascontiguousarray(x[b].T).astype(bf) for b in range(B)]
    in_maps = []
    for c in range(B * NG):
        b, g = c % B, c // B
        qs = slice(g * FG, (g + 1) * FG)
        ks = slice(E + g * FG, E + (g + 1) * FG)
        vs = slice(2 * E + g * FG, 2 * E + (g + 1) * FG)
        wqkT = np.ascontiguousarray(
            np.concatenate([W_qkv[qs], W_qkv[ks]], axis=0).T
        ).astype(bf)
        wvT = np.ascontiguousarray(W_qkv[vs].T).astype(bf)
        woutT = np.ascontiguousarray(W_out[:, qs].T).astype(bf)
        in_maps.append(
            {
                "xT": xTs[b],
                "wqkT": wqkT,
                "wvT": wvT,
                "woutT": woutT,
                "cos2T": cos2T,
                "sin2T": sin2T,
                "p2": p2,
                "ones": ones,
            }
        )
    return in_maps


_NC_CACHE = {}


def _get_nc(cfg_key):
    if cfg_key not in _NC_CACHE:
        _NC_CACHE[cfg_key] = _build(
            dict(zip(("S", "E", "HG", "pump_qi", "pump_hp3"), cfg_key))
        )
    return _NC_CACHE[cfg_key]


def kernel(x, cos, sin, W_qkv, W_out, _trace=False):
    x = np.asarray(x, dtype=np.float32)
    cos = np.asarray(cos, dtype=np.float32)
    sin = np.asarray(sin, dtype=np.float32)
    W_qkv = np.asarray(W_qkv, dtype=np.float32)
    W_out = np.asarray(W_out, dtype=np.float32)
    B, S, E = x.shape
    pump_qi = int(os.environ.get("K_PUMP_QI", "8"))
    pump_hp3 = int(os.environ.get("K_PUMP_HP3", "2"))
    cfg = dict(S=S, E=E, HG=8, pump_qi=pump_qi, pump_hp3=pump_hp3)
    nc = _get_nc((S, E, 8, pump_qi, pump_hp3))
    in_maps = make_core_inputs(x, cos, sin, W_qkv, W_out, cfg)

    from concourse.bass_utils import run_bass_kernel_spmd

    res = run_bass_kernel_spmd(
        nc, in_maps, core_ids=list(range(8)), trace=_trace
    )
    outs = [np.asarray(r["outT"], dtype=np.float32) for r in res.results]
    out = np.empty((B, S, E), dtype=np.float32)
    for b in range(B):
        out[b] = (outs[b] + outs[b + B]).T
    kernel.last_result = res
    return out


# revision 24
# speedup vs baseline: 1.5608x; 1.5608x over previous
"""Trainium2 Bass kernel for nn_MHAEncoderFusedProj.

B=4, S=2048, E=1024, H=16, D=64. Sharding: 8 cores = 4 batch x 2
head-groups (8 heads each). No collectives: each core computes a partial
out-projection over its 512 o-features; the host adds the two partials per
batch element and transposes back.

v2 design (all-bf16 data path, ACT-paced attention with PE fillers):
  - All matmul operands are bf16 (1 col/cycle PE streaming vs ~2 for
    fp32/fp32r). PSUM accumulation stays fp32.
  - x^T is loaded into SBUF once (bf16) and reused by the QKV projections.
  - Attention inner loop per (pair, q-chunk, kt): score pair packed into PE
    row groups (runs concurrently), exp on ScalarE (the pacer at ~1.1us per
    [128,1024] tile), PV accumulation with a ones-column appended to V so
    row 64 is the softmax denominator.
  - The ScalarE exp stream leaves ~300-400ns/iter of PE slack; projection,
    RoPE and out-projection matmuls for the *next* head-pair are emitted as
    fine-grained "fillers" (one matmul per kt slot) so they hide inside the
    attention window instead of extending the span.
  - Normalization: reciprocal of the denominator row, gpsimd partition
    broadcast, DVE multiply (second head's half moved by SBUF-SBUF DMA).
"""

import math
import os

import numpy as np

P = 128
D = 64

FULL_CFG = dict(S=2048, E=1024, HG=8)


def _emit(nc, tc, io, cfg):
    import concourse.mybir as mybir

    FP32 = mybir.dt.float32
    BF16 = mybir.dt.bfloat16
    EXP = mybir.ActivationFunctionType.Exp
    CPY = mybir.ActivationFunctionType.Copy

    S, E, HG = cfg["S"], cfg["E"], cfg["HG"]
    EO = E // P              # contraction tiles over embedding
    MQK = 2 * HG * D // P    # Q+K feature tiles (2 heads per tile)
    NPAIR = HG // 2          # head pairs per core
    FV = HG * D              # V features
    KT = S // P              # key token tiles
    CH = 512                 # chunk for projections / rope / out-proj
    NTA = S // CH
    QCH = 512                # q chunk in attention
    NQI = S // QCH
    FO = E // P              # out-proj feature tiles
    EOV = FV // P            # out-proj contraction tiles
    scale = 1.0 / math.sqrt(D)
    PUMP_QI = cfg.get("pump_qi", 8)
    PUMP_HP3 = cfg.get("pump_hp3", 2)

    xT = io["xT"].ap()          # [E, S] bf16
    wqkT = io["wqkT"].ap()      # [E, 2*HG*D] bf16
    wvT = io["wvT"].ap()        # [E, HG*D] bf16
    woutT = io["woutT"].ap()    # [HG*D, E] bf16
    cos2T = io["cos2T"].ap()    # [P, S] bf16
    sin2T = io["sin2T"].ap()    # [P, S] fp32
    p2 = io["p2"].ap()          # [P, P] bf16 signed rotate-half permutation
    ones = io["ones"]           # [P, KT*HG] bf16
    outT = io["outT"].ap()      # [E, S] bf16

    xT_t = xT.rearrange("(eo p) t -> p eo t", p=P)
    outT_t = outT.rearrange("(fo p) t -> p fo t", p=P)

    from contextlib import ExitStack

    with ExitStack() as top:
        persist = top.enter_context(tc.tile_pool(name="persist", bufs=1))
        # PSUM budget (8 banks): scores ping/pong 2x[128,1024] = 4 banks,
        # PV accumulators 2x[128,512] = 2 banks, filler (proj/rope/B/D)
        # 2x[128,512] = 2 banks.
        psc = top.enter_context(tc.tile_pool(name="psc", bufs=2, space="PSUM"))
        ppv = top.enter_context(tc.tile_pool(name="ppv", bufs=1, space="PSUM"))
        pfill = top.enter_context(tc.tile_pool(name="pfill", bufs=1, space="PSUM"))
        prope = top.enter_context(tc.tile_pool(name="prope", bufs=1, space="PSUM"))
        ep = top.enter_context(tc.tile_pool(name="ep", bufs=4))
        tap = top.enter_context(tc.tile_pool(name="ropeT", bufs=1))
        npool = top.enter_context(tc.tile_pool(name="norm", bufs=2))
        opool = top.enter_context(tc.tile_pool(name="opool", bufs=2))
        dcp = top.enter_context(tc.tile_pool(name="dcp", bufs=2))

        xsb = persist.tile([P, EO, S], BF16, tag="xsb")
        wqk = persist.tile([P, EO, MQK * P], BF16, tag="wqk")
        wv = persist.tile([P, EO, FV], BF16, tag="wv")
        wo = persist.tile([P, EOV, E], BF16, tag="wo")
        vsb = persist.tile([P, KT, HG, D + 1], BF16, tag="vsb")
        qk = [persist.tile([P, S], BF16, tag=f"qk{m}", name=f"qk{m}") for m in range(MQK)]
        ost = [persist.tile([P, S], BF16, tag=f"ost{j}", name=f"ost{j}") for j in range(NPAIR)]
        cosb = persist.tile([P, S], BF16, tag="cosb")
        sinb = persist.tile([P, S], FP32, tag="sinb")
        p2b = persist.tile([P, P], BF16, tag="p2b")

        # upfront DMAs, split across the two HWDGE queues (sync + scalar):
        # x and wqk ride the scalar queue, everything else the sync queue.
        nc.sync.dma_start(wv, wvT.rearrange("(eo p) f -> p eo f", p=P))
        XD = 256
        for c in range(S // XD):
            eng = nc.scalar if c % 2 == 0 else nc.sync
            eng.dma_start(
                xsb[:, :, c * XD : (c + 1) * XD], xT_t[:, :, c * XD : (c + 1) * XD]
            )
        nc.scalar.dma_start(wqk, wqkT.rearrange("(eo p) f -> p eo f", p=P))
        nc.sync.dma_start(vsb[:, :, :, D : D + 1], ones.ap())
        nc.sync.dma_start(cosb, cos2T)
        nc.sync.dma_start(sinb, sin2T)
        nc.sync.dma_start(p2b, p2)
        nc.sync.dma_start(wo, woutT.rearrange("(eo p) f -> p eo f", p=P))

        # ---------------- filler machinery ----------------
        fill_q = []
        pending_norm = []

        def pump(n):
            for _ in range(n):
                while fill_q:
                    try:
                        next(fill_q[0])()
                        break
                    except StopIteration:
                        fill_q.pop(0)
                else:
                    return

        def drain():
            while fill_q:
                try:
                    next(fill_q[0])()
                except StopIteration:
                    fill_q.pop(0)

        # ---------------- phase emitters ----------------
        def b_unit(tt, on_act=False):
            """V projection for token tile tt (all 8 heads)."""
            ps = pfill.tile([P, FV], FP32, tag="fill", name="psB")
            for e in range(EO):
                nc.tensor.matmul(
                    ps,
                    xsb[:, e, tt * P : (tt + 1) * P],
                    wv[:, e, :],
                    start=(e == 0),
                    stop=(e == EO - 1),
                )
            srcv = ps.rearrange("p (h d) -> p h d", d=D)
            if on_act:
                nc.scalar.activation(vsb[:, tt, :, 0:D], srcv, CPY)
            else:
                nc.vector.tensor_copy(vsb[:, tt, :, 0:D], srcv)

        def a_unit_gen(m, ta, on_act=False):
            """QK projection chunk + RoPE, one PE op per yield."""
            sl = slice(ta * CH, (ta + 1) * CH)
            ps = pfill.tile([P, CH], FP32, tag="fill", name="psA")
            for e in range(EO):
                yield (
                    lambda ps=ps, m=m, e=e, sl=sl: nc.tensor.matmul(
                        ps,
                        wqk[:, e, m * P : (m + 1) * P],
                        xsb[:, e, sl],
                        start=(e == 0),
                        stop=(e == EO - 1),
                    )
                )

            def cp(ps=ps, m=m, sl=sl):
                if on_act:
                    nc.scalar.activation(qk[m][:, sl], ps, CPY)
                else:
                    nc.vector.tensor_copy(qk[m][:, sl], ps)

            yield cp

            def rope(m=m, sl=sl):
                rps = prope.tile([P, CH], FP32, tag="rps", name="rps")
                nc.tensor.matmul(rps, p2b, qk[m][:, sl], start=True, stop=True)
                t1 = tap.tile([P, CH], BF16, tag="t1")
                nc.vector.tensor_mul(t1, qk[m][:, sl], cosb[:, sl])
                t2 = tap.tile([P, CH], BF16, tag="t2")
                nc.vector.tensor_mul(t2, rps, sinb[:, sl])
                nc.vector.tensor_add(qk[m][:, sl], t1, t2)

            yield rope

        def a_pair_gen(hp):
            """QK projection + RoPE for head pair hp, one PE op per yield."""
            for ta in range(NTA):
                for m in (hp, NPAIR + hp):
                    yield from a_unit_gen(m, ta, on_act=False)

        def d_gen(td):
            """Partial out-projection for token chunk td (needs all pairs)."""
            sl = slice(td * CH, (td + 1) * CH)
            for fo in range(FO):
                ps = pfill.tile([P, CH], FP32, tag="fill", name="psD")
                for e in range(EOV):
                    yield (
                        lambda ps=ps, fo=fo, e=e, sl=sl: nc.tensor.matmul(
                            ps,
                            wo[:, e, fo * P : (fo + 1) * P],
                            ost[e][:, sl],
                            start=(e == 0),
                            stop=(e == EOV - 1),
                        )
                    )

                def cpd(ps=ps, fo=fo, sl=sl):
                    ot = dcp.tile([P, CH], BF16, tag="ot")
                    nc.vector.tensor_copy(ot, ps)
                    nc.gpsimd.dma_start(outT_t[:, fo, sl], ot)

                yield cpd

        def attention_pair(hp, pump_n):
            qt = qk[hp]
            ktile = qk[NPAIR + hp]
            for qi in range(NQI):
                qsl = slice(qi * QCH, (qi + 1) * QCH)
                ops = [
                    ppv.tile([P, QCH], FP32, tag=f"pv{hs}", name=f"ops{hs}")
                    for hs in range(2)
                ]
                exs = {}

                def sc_exp(kt):
                    scps = psc.tile([P, 2 * QCH], FP32, tag="sc", name="scps")
                    ksl = slice(kt * P, (kt + 1) * P)
                    for hs in range(2):
                        b = hs * D
                        nc.tensor.matmul(
                            scps[:, hs * QCH : (hs + 1) * QCH],
                            ktile[b : b + D, ksl],
                            qt[b : b + D, qsl],
                            start=True,
                            stop=True,
                        )
                    ex = ep.tile([P, 2 * QCH], BF16, tag="exp")
                    nc.scalar.activation(ex, scps, EXP, scale=scale)
                    exs[kt] = ex

                def pv(kt):
                    ex = exs.pop(kt)
                    for hs in range(2):
                        nc.tensor.matmul(
                            ops[hs][0 : D + 1, :],
                            vsb[:, kt, 2 * hp + hs, :],
                            ex[:, hs * QCH : (hs + 1) * QCH],
                            start=(kt == 0),
                            stop=(kt == KT - 1),
                        )

                sc_exp(0)
                for kt in range(1, KT):
                    sc_exp(kt)
                    if kt % 3 == 0 and pending_norm:
                        try:
                            next(pending_norm[0])
                        except StopIteration:
                            pending_norm.pop(0)
                    pump(pump_n)
                    pv(kt - 1)
                pv(KT - 1)

                # free the PV PSUM banks immediately; the rest of the
                # normalization is staggered over the next qi's kt slots so
                # no DVE instruction ever waits on a cross-engine chain.
                ostas = []
                for hs in range(2):
                    osta = opool.tile([D + 1, QCH], FP32, tag=f"osta{hs}")
                    nc.vector.tensor_copy(osta, ops[hs][0 : D + 1, :])
                    ostas.append(osta)

                def norm_steps(hp=hp, qsl=qsl, ostas=ostas):
                    rs8s, rrs, rbcs = [], [], []
                    for hs in range(2):
                        rs8 = npool.tile([P, QCH // P], FP32, tag=f"rs8{hs}")
                        nc.sync.dma_start(rs8, ostas[hs][D : D + 1, :])
                        rs8s.append(rs8)
                    yield
                    for hs in range(2):
                        ri8 = npool.tile([P, QCH // P], FP32, tag=f"ri8{hs}")
                        nc.vector.reciprocal(ri8, rs8s[hs])
                        rr = npool.tile([1, QCH], FP32, tag=f"rr{hs}")
                        nc.sync.dma_start(rr, ri8)
                        rrs.append(rr)
                    yield
                    for hs in range(2):
                        rbc = npool.tile([D, QCH], FP32, tag=f"rbc{hs}")
                        nc.gpsimd.partition_broadcast(rbc, rrs[hs])
                        rbcs.append(rbc)
                    yield
                    nc.vector.tensor_mul(ost[hp][0:D, qsl], ostas[0][0:D, :], rbcs[0])
                    otmp = npool.tile([D, QCH], BF16, tag="otmp")
                    nc.vector.tensor_mul(otmp, ostas[1][0:D, :], rbcs[1])
                    nc.sync.dma_start(ost[hp][D : 2 * D, qsl], otmp)

                while pending_norm:           # at most one qi of lag
                    for _ in pending_norm.pop(0):
                        pass
                if hp == NPAIR - 1:
                    # D reads ost: the norm chain must be fully emitted
                    # before d_gen yields can be pumped
                    for _ in norm_steps():
                        pass
                    fill_q.append(d_gen(qi))
                else:
                    pending_norm.append(norm_steps())
                pump(PUMP_QI)

        # ---------------- prologue: V proj + pair-0 QK/rope ----------------
        # Copies go on ScalarE (idle before attention) so the DVE never
        # develops a backlog that would stall attention-era filler matmuls.
        for c in range(NTA):
            for tt in range(4 * c, 4 * c + 4):
                b_unit(tt, on_act=True)
            for f in a_unit_gen(0, c, on_act=True):
                f()
            for f in a_unit_gen(NPAIR, c, on_act=True):
                f()

        # ---------------- attention with interleaved fillers ----------------
        for hp in range(NPAIR):
            if hp < NPAIR - 1:
                fill_q.append(a_pair_gen(hp + 1))
            attention_pair(hp, PUMP_HP3 if hp == NPAIR - 1 else 1)
            if hp < NPAIR - 1:
                drain()  # pair hp+1 must be fully projected+roped
        drain()  # remaining out-projection chunks


def _build(cfg):
    from concourse import bacc
    import concourse.mybir as mybir
    import concourse.tile as tile

    S, E, HG = cfg["S"], cfg["E"], cfg["HG"]
    FP32 = mybir.dt.float32
    BF16 = mybir.dt.bfloat16
    nc = bacc.Bacc("TRN2", target_bir_lowering=False, debug=False)
    io = {
        "xT": nc.dram_tensor("xT", [E, S], BF16, kind="ExternalInput"),
        "wqkT": nc.dram_tensor("wqkT", [E, 2 * HG * D], BF16, kind="ExternalInput"),
        "wvT": nc.dram_tensor("wvT", [E, HG * D], BF16, kind="ExternalInput"),
        "woutT": nc.dram_tensor("woutT", [HG * D, E], BF16, kind="ExternalInput"),
        "cos2T": nc.dram_tensor("cos2T", [P, S], BF16, kind="ExternalInput"),
        "sin2T": nc.dram_tensor("sin2T", [P, S], FP32, kind="ExternalInput"),
        "p2": nc.dram_tensor("p2", [P, P], BF16, kind="ExternalInput"),
        "ones": nc.dram_tensor(
            "ones", [P, (S // P) * HG], BF16, kind="ExternalInput"
        ),
        "outT": nc.dram_tensor("outT", [E, S], BF16, kind="ExternalOutput"),
    }
    with tile.TileContext(nc) as tc:
        _emit(nc, tc, io, cfg)
    nc.compile()
    return nc


def _rot_matrix():
    """P2[p, m] such that (P2^T @ v) = rotate_half(v) for the 2-head
    [128]-row layout (two independent 64-blocks)."""
    p2 = np.zeros((P, P), dtype=np.float32)
    for blk in (0, 64):
        for d in range(32):
            p2[blk + d + 32, blk + d] = -1.0
            p2[blk + d, blk + d + 32] = 1.0
    return p2


def make_core_inputs(x, cos, sin, W_qkv, W_out, cfg=FULL_CFG):
    """Host-side shard prep. Returns list of 8 in_maps."""
    import ml_dtypes

    bf = ml_dtypes.bfloat16
    S, E, HG = cfg["S"], cfg["E"], cfg["HG"]
    B = x.shape[0]
    NG = 2  # head groups
    FG = HG * D  # features per group
    cos2T = np.ascontiguousarray(np.tile(cos.T, (2, 1))).astype(bf)
    sin2T = np.ascontiguousarray(np.tile(sin.T, (2, 1))).astype(np.float32)
    p2 = _rot_matrix().astype(bf)
    ones = np.ones((P, (S // P) * HG), dtype=bf)
    xTs = [np.ascontiguousarray(x[b].T).astype(bf) for b in range(B)]
    in_maps = []
    for c in range(B * NG):
        b, g = c % B, c // B
        qs = slice(g * FG, (g + 1) * FG)
        ks = slice(E + g * FG, E + (g + 1) * FG)
        vs = slice(2 * E + g * FG, 2 * E + (g + 1) * FG)
        wqkT = np.ascontiguousarray(
            np.concatenate([W_qkv[qs], W_qkv[ks]], axis=0).T
        ).astype(bf)
        wvT = np.ascontiguousarray(W_qkv[vs].T).astype(bf)
        woutT = np.ascontiguousarray(W_out[:, qs].T).astype(bf)
        in_maps.append(
            {
                "xT": xTs[b],
                "wqkT": wqkT,
                "wvT": wvT,
                "woutT": woutT,
                "cos2T": cos2T,
                "sin2T": sin2T,
                "p2": p2,
                "ones": ones,
            }
        )
    return in_maps


_NC_CACHE = {}


def _get_nc(cfg_key):
    if cfg_key not in _NC_CACHE:
        _NC_CACHE[cfg_key] = _build(
            dict(zip(("S", "E", "HG", "pump_qi", "pump_hp3"), cfg_key))
        )
    return _NC_CACHE[cfg_key]


def kernel(x, cos, sin, W_qkv, W_out, _trace=False):
    x = np.asarray(x, dtype=np.float32)
    cos = np.asarray(cos, dtype=np.float32)
    sin = np.asarray(sin, dtype=np.float32)
    W_qkv = np.asarray(W_qkv, dtype=np.float32)
    W_out = np.asarray(W_out, dtype=np.float32)
    B, S, E = x.shape
    pump_qi = int(os.environ.get("K_PUMP_QI", "8"))
    pump_hp3 = int(os.environ.get("K_PUMP_HP3", "2"))
    cfg = dict(S=S, E=E, HG=8, pump_qi=pump_qi, pump_hp3=pump_hp3)
    nc = _get_nc((S, E, 8, pump_qi, pump_hp3))
    in_maps = make_core_inputs(x, cos, sin, W_qkv, W_out, cfg)

    from concourse.bass_utils import run_bass_kernel_spmd

    res = run_bass_kernel_spmd(
        nc, in_maps, core_ids=list(range(8)), trace=_trace
    )
    outs = [np.asarray(r["outT"], dtype=np.float32) for r in res.results]
    out = np.empty((B, S, E), dtype=np.float32)
    for b in range(B):
        out[b] = (outs[b] + outs[b + B]).T
    kernel.last_result = res
    return out
